# revision 5
# baseline (speedup 1.0000x reference)
"""nn_MergeWindows — Trainium2 Bass kernel (8 NeuronCores, SPMD over image rows).

Key observation: the reference's sequential merge scan over candidate channel
pairs depends only on tiny metadata — per-channel edge-touch bits along the
window boundaries (rows/cols 511/512 of the 1024x1024 image) and cosine sims
of the [4,7,64] slot features.  The final output is exactly

    out[b, c, y, x] = 1.0  iff  remap[argmax_d masks[b, d, y, x]] == c

where remap: [32]->[32] merges channels per the scan.  remap is computed on
the host (numpy, microseconds — it reads 4 boundary strips), and the heavy
per-pixel work (argmax over 32 channels + one-hot, 128 MiB in) runs on 8
NeuronCores, each handling 128 of the 1024 rows.

Device pipeline per [128 rows, 32 ch, 256 cols] tile (pixel-major layout,
rows on partitions), all on the vector engine, contiguous unit-stride APs:
  1. 5-step max tree over the channel axis -> mx [128, 256]
     (tensor_tensor max halvings: 16+8+4+2+1; a strided tensor_reduce over
     the channel axis measures 2.4x slower than this tree)
  2. eq = is_equal(masks, mx broadcast over channels) -> bf16 one-hot
     (f32 max returns one input bit-exactly, so eq == one_hot(argmax) except
     at the handful of pixels where two channels are bit-identical; those
     tie pixels are detected and patched on the host)
  3. DMA eq out (bf16: halves output HBM traffic; 0/1 is exact in bf16)

Host post-processing (numpy, ~100 ms): detect tie pixels (channel-sum != 1),
re-argmax just those pixels, apply the merge remap as channel-plane OR/zero
ops, cast to f32.  This keeps the device program input-independent (single
cached compile) and the device DMA-bound at ~25 MiB per core.
"""

import json

import numpy as np

N_WINDOWS = 4
WIN_H = WIN_W = 512
IMG_H = IMG_W = 1024
C = 32
MPW = C // N_WINDOWS
SLOT_DIM = 64
SIM_THRESH = 0.1

N_CORES = 8
ROWS_PER_CORE = IMG_H // N_CORES  # 128
G = 512          # column-tile width
NTILES = IMG_W // G

_cache = {}


# --------------------------------------------------------------------------
# host-side merge decision (mirrors reference._merge_windows metadata math)
# --------------------------------------------------------------------------
def _compute_remap(masks, slot_features, pl, pt):
    B, Ch, H, W = masks.shape
    mpw = Ch // N_WINDOWS
    ranges = [(i * mpw, (i + 1) * mpw) for i in range(N_WINDOWS)]

    adjacency = []
    for i in range(N_WINDOWS):
        for j in range(i + 1, N_WINDOWS):
            if pt[i] == pt[j] and abs(pl[i] - pl[j]) == WIN_W:
                adjacency.append((i, j, True) if pl[i] < pl[j] else (j, i, True))
            if pl[i] == pl[j] and abs(pt[i] - pt[j]) == WIN_H:
                adjacency.append((i, j, False) if pt[i] < pt[j] else (j, i, False))

    edge_l = np.zeros(Ch, bool)
    edge_r = np.zeros(Ch, bool)
    edge_t = np.zeros(Ch, bool)
    edge_b = np.zeros(Ch, bool)
    m0 = masks[0]
    for wi, (s, e) in enumerate(ranges):
        ys, ye = max(pt[wi], 0), min(pt[wi] + WIN_H, H)
        xs, xe = max(pl[wi], 0), min(pl[wi] + WIN_W, W)
        if ys >= ye or xs >= xe:
            continue
        ids_l = np.argmax(m0[:, ys:ye, xs], axis=0)
        ids_r = np.argmax(m0[:, ys:ye, xe - 1], axis=0)
        ids_t = np.argmax(m0[:, ys, xs:xe], axis=0)
        ids_b = np.argmax(m0[:, ye - 1, xs:xe], axis=0)
        for k in range(s, e):
            edge_l[k] = np.any(ids_l == k)
            edge_r[k] = np.any(ids_r == k)
            edge_t[k] = np.any(ids_t == k)
            edge_b[k] = np.any(ids_b == k)

    ci_l, cj_l, wi_l, wj_l, hz_l = [], [], [], [], []
    for wi, wj, horiz in adjacency:
        si, ei = ranges[wi]
        sj, ej = ranges[wj]
        for ci in range(si + 1, ei):
            for cj in range(sj + 1, ej):
                ci_l.append(ci)
                cj_l.append(cj)
                wi_l.append(wi)
                wj_l.append(wj)
                hz_l.append(horiz)

    target = np.arange(Ch)
    if not ci_l:
        return target

    sf = np.asarray(slot_features, np.float32)
    sf_n = sf / (np.linalg.norm(sf, axis=-1, keepdims=True) + np.float32(1e-8))
    ci_a = np.array(ci_l)
    cj_a = np.array(cj_l)
    rel_i = ci_a % mpw - 1
    rel_j = cj_a % mpw - 1
    fi = sf_n[np.array(wi_l), rel_i]
    fj = sf_n[np.array(wj_l), rel_j]
    sims = np.sum(fi * fj, axis=-1)
    hz = np.array(hz_l)
    edge_ok = np.where(hz, edge_r[ci_a] & edge_l[cj_a], edge_b[ci_a] & edge_t[cj_a])
    passing = edge_ok & (sims > np.float32(SIM_THRESH))

    merged = np.zeros(Ch, bool)
    for ci, cj, ok in zip(ci_l, cj_l, passing):
        if ok and not merged[ci] and not merged[cj]:
            keep, rem = min(ci, cj), max(ci, cj)
            target[target == rem] = keep
            merged[rem] = True
    return target


# --------------------------------------------------------------------------
# wait-split post-pass: the pinned neuronxcc allows only ONE sync wait per
# instruction; hoist extras onto preceding same-engine EventSemaphore insts.
# --------------------------------------------------------------------------
def _split_excess_waits(bir_json_bytes, limit=1):
    j = json.loads(bir_json_bytes)
    counter = [0]
    for fn in j.get("functions", []):
        for bb in fn.get("blocks", []):
            new_insts = []
            for inst in bb.get("instructions", []):
                si = inst.get("sync_info") or {}
                waits = si.get("on_wait") or []
                if len(waits) > limit:
                    extra = waits[: len(waits) - limit]
                    si["on_wait"] = waits[len(waits) - limit:]
                    inst["sync_info"] = si
                    for i in range(0, len(extra), limit):
                        counter[0] += 1
                        new_insts.append({
                            "engine": inst["engine"],
                            "ins": [],
                            "name": f"{inst['name']}_hoistw{counter[0]}",
                            "opcode": "EventSemaphore",
                            "outs": [],
                            "sync_info": {"on_update": [],
                                          "on_wait": extra[i: i + limit]},
                        })
                new_insts.append(inst)
            bb["instructions"] = new_insts
    return json.dumps(j).encode()


def _build_program():
    if "prog" in _cache:
        return _cache["prog"]

    import concourse.bass as bass
    import concourse.tile as tile
    from concourse import mybir

    f32 = mybir.dt.float32
    u8 = mybir.dt.uint8
    nc = bass.Bass()
    masks_in = nc.dram_tensor("masks", [C, ROWS_PER_CORE, IMG_W], f32,
                              kind="ExternalInput")
    out_dram = nc.dram_tensor("out", [C, ROWS_PER_CORE, IMG_W], u8,
                              kind="ExternalOutput")

    with tile.TileContext(nc) as tc:
        with (
            tc.tile_pool(name="inp", bufs=2) as inp,
            tc.tile_pool(name="outp", bufs=2) as outp,
            tc.tile_pool(name="scr", bufs=1) as scr,
        ):
            for t in range(NTILES):
                sl = slice(G * t, G * (t + 1))
                in_tile = inp.tile([128, C, G], f32, tag="in_tile")
                nc.sync.dma_start(
                    in_tile[:], masks_in[:, :, sl].rearrange("d p g -> p d g"))

                # 5-step contiguous max tree over the channel axis
                s = scr.tile([128, 16, G], f32, tag="tree")
                nc.vector.tensor_tensor(
                    out=s[:], in0=in_tile[:, 0:16, :], in1=in_tile[:, 16:32, :],
                    op=mybir.AluOpType.max)
                for h in (8, 4, 2, 1):
                    nc.vector.tensor_tensor(
                        out=s[:, 0:h, :], in0=s[:, 0:h, :], in1=s[:, h:2 * h, :],
                        op=mybir.AluOpType.max)

                # one-hot: compare every channel against the broadcast max
                eq = outp.tile([128, C, G], u8, tag="eq")
                mx_ap = s[:, 0, :]
                mx_b = bass.AP(tensor=mx_ap.tensor, offset=mx_ap.offset,
                               ap=[mx_ap.ap[0], [0, C], mx_ap.ap[-1]])
                nc.vector.tensor_tensor(out=eq[:], in0=in_tile[:], in1=mx_b,
                                        op=mybir.AluOpType.is_equal)

                nc.sync.dma_start(
                    out_dram[:, :, sl].rearrange("c p g -> p c g"), eq[:])

    orig = nc.to_json_bytes
    nc.to_json_bytes = lambda: _split_excess_waits(orig())
    _cache["prog"] = nc
    return nc


def kernel(masks, slot_features, pad_left, pad_top):
    from concourse.bass_utils import run_bass_kernel_spmd

    masks = np.asarray(masks, np.float32)
    slot_features = np.asarray(slot_features, np.float32)
    pl = [int(v) for v in np.asarray(pad_left)]
    pt = [int(v) for v in np.asarray(pad_top)]

    remap = _compute_remap(masks, slot_features, pl, pt)

    nc = _build_program()
    in_maps = []
    for i in range(N_CORES):
        slab = np.ascontiguousarray(
            masks[0, :, i * ROWS_PER_CORE:(i + 1) * ROWS_PER_CORE, :])
        in_maps.append({"masks": slab})

    res = run_bass_kernel_spmd(nc, in_maps, core_ids=list(range(N_CORES)))

    # assemble the per-core bf16 one-hots as booleans
    oh = np.empty((C, IMG_H, IMG_W), np.bool_)
    for i, r in enumerate(res.results):
        arr = np.asarray(r["out"])
        nz = arr.view(np.uint16) != 0 if arr.itemsize == 2 else arr != 0
        oh[:, i * ROWS_PER_CORE:(i + 1) * ROWS_PER_CORE, :] = nz

    # pixels where two channels tied bit-exactly produced two 1s; find them
    # before the merge pass and patch from the raw input afterwards
    colsum = oh.sum(axis=0, dtype=np.int16)
    ties = np.argwhere(colsum != 1)

    # merge remap as channel-plane ops (exactly the reference's add+zero scan)
    for d in range(C):
        k = int(remap[d])
        if k != d:
            oh[k] |= oh[d]
            oh[d] = False

    for y, x in ties:
        w = int(np.argmax(masks[0, :, y, x]))
        oh[:, y, x] = False
        oh[int(remap[w]), y, x] = True

    return oh.astype(np.float32)[None]


# revision 6
# speedup vs baseline: 1.1728x; 1.1728x over previous
"""nn_MergeWindows — Trainium2 Bass kernel (8 NeuronCores, SPMD over image rows).

Key observation: the reference's sequential merge scan over candidate channel
pairs depends only on tiny metadata — per-channel edge-touch bits along the
window boundaries (rows/cols 511/512 of the 1024x1024 image) and cosine sims
of the [4,7,64] slot features.  The final output is exactly

    out[b, c, y, x] = 1.0  iff  remap[argmax_d masks[b, d, y, x]] == c

where remap: [32]->[32] merges channels per the scan.  remap is computed on
the host (numpy, microseconds — it reads 4 boundary strips), and the heavy
per-pixel work (argmax over 32 channels + one-hot, 128 MiB in) runs on 8
NeuronCores, each handling 128 of the 1024 rows.

Device pipeline per [128 rows, 32 ch, 256 cols] tile (pixel-major layout,
rows on partitions), all on the vector engine, contiguous unit-stride APs:
  1. 5-step max tree over the channel axis -> mx [128, 256]
     (tensor_tensor max halvings: 16+8+4+2+1; a strided tensor_reduce over
     the channel axis measures 2.4x slower than this tree)
  2. eq = is_equal(masks, mx broadcast over channels) -> bf16 one-hot
     (f32 max returns one input bit-exactly, so eq == one_hot(argmax) except
     at the handful of pixels where two channels are bit-identical; those
     tie pixels are detected and patched on the host)
  3. DMA eq out (bf16: halves output HBM traffic; 0/1 is exact in bf16)

Host post-processing (numpy, ~100 ms): detect tie pixels (channel-sum != 1),
re-argmax just those pixels, apply the merge remap as channel-plane OR/zero
ops, cast to f32.  This keeps the device program input-independent (single
cached compile) and the device DMA-bound at ~25 MiB per core.
"""

import json

import numpy as np

N_WINDOWS = 4
WIN_H = WIN_W = 512
IMG_H = IMG_W = 1024
C = 32
MPW = C // N_WINDOWS
SLOT_DIM = 64
SIM_THRESH = 0.1

N_CORES = 8
ROWS_PER_CORE = IMG_H // N_CORES  # 128
G = 256          # column-tile width
NTILES = IMG_W // G

_cache = {}


# --------------------------------------------------------------------------
# host-side merge decision (mirrors reference._merge_windows metadata math)
# --------------------------------------------------------------------------
def _compute_remap(masks, slot_features, pl, pt):
    B, Ch, H, W = masks.shape
    mpw = Ch // N_WINDOWS
    ranges = [(i * mpw, (i + 1) * mpw) for i in range(N_WINDOWS)]

    adjacency = []
    for i in range(N_WINDOWS):
        for j in range(i + 1, N_WINDOWS):
            if pt[i] == pt[j] and abs(pl[i] - pl[j]) == WIN_W:
                adjacency.append((i, j, True) if pl[i] < pl[j] else (j, i, True))
            if pl[i] == pl[j] and abs(pt[i] - pt[j]) == WIN_H:
                adjacency.append((i, j, False) if pt[i] < pt[j] else (j, i, False))

    edge_l = np.zeros(Ch, bool)
    edge_r = np.zeros(Ch, bool)
    edge_t = np.zeros(Ch, bool)
    edge_b = np.zeros(Ch, bool)
    m0 = masks[0]
    for wi, (s, e) in enumerate(ranges):
        ys, ye = max(pt[wi], 0), min(pt[wi] + WIN_H, H)
        xs, xe = max(pl[wi], 0), min(pl[wi] + WIN_W, W)
        if ys >= ye or xs >= xe:
            continue
        ids_l = np.argmax(m0[:, ys:ye, xs], axis=0)
        ids_r = np.argmax(m0[:, ys:ye, xe - 1], axis=0)
        ids_t = np.argmax(m0[:, ys, xs:xe], axis=0)
        ids_b = np.argmax(m0[:, ye - 1, xs:xe], axis=0)
        for k in range(s, e):
            edge_l[k] = np.any(ids_l == k)
            edge_r[k] = np.any(ids_r == k)
            edge_t[k] = np.any(ids_t == k)
            edge_b[k] = np.any(ids_b == k)

    ci_l, cj_l, wi_l, wj_l, hz_l = [], [], [], [], []
    for wi, wj, horiz in adjacency:
        si, ei = ranges[wi]
        sj, ej = ranges[wj]
        for ci in range(si + 1, ei):
            for cj in range(sj + 1, ej):
                ci_l.append(ci)
                cj_l.append(cj)
                wi_l.append(wi)
                wj_l.append(wj)
                hz_l.append(horiz)

    target = np.arange(Ch)
    if not ci_l:
        return target

    sf = np.asarray(slot_features, np.float32)
    sf_n = sf / (np.linalg.norm(sf, axis=-1, keepdims=True) + np.float32(1e-8))
    ci_a = np.array(ci_l)
    cj_a = np.array(cj_l)
    rel_i = ci_a % mpw - 1
    rel_j = cj_a % mpw - 1
    fi = sf_n[np.array(wi_l), rel_i]
    fj = sf_n[np.array(wj_l), rel_j]
    sims = np.sum(fi * fj, axis=-1)
    hz = np.array(hz_l)
    edge_ok = np.where(hz, edge_r[ci_a] & edge_l[cj_a], edge_b[ci_a] & edge_t[cj_a])
    passing = edge_ok & (sims > np.float32(SIM_THRESH))

    merged = np.zeros(Ch, bool)
    for ci, cj, ok in zip(ci_l, cj_l, passing):
        if ok and not merged[ci] and not merged[cj]:
            keep, rem = min(ci, cj), max(ci, cj)
            target[target == rem] = keep
            merged[rem] = True
    return target


# --------------------------------------------------------------------------
# wait-split post-pass: the pinned neuronxcc allows only ONE sync wait per
# instruction; hoist extras onto preceding same-engine EventSemaphore insts.
# --------------------------------------------------------------------------
def _split_excess_waits(bir_json_bytes, limit=1):
    j = json.loads(bir_json_bytes)
    counter = [0]
    for fn in j.get("functions", []):
        for bb in fn.get("blocks", []):
            new_insts = []
            for inst in bb.get("instructions", []):
                si = inst.get("sync_info") or {}
                waits = si.get("on_wait") or []
                if len(waits) > limit:
                    extra = waits[: len(waits) - limit]
                    si["on_wait"] = waits[len(waits) - limit:]
                    inst["sync_info"] = si
                    for i in range(0, len(extra), limit):
                        counter[0] += 1
                        new_insts.append({
                            "engine": inst["engine"],
                            "ins": [],
                            "name": f"{inst['name']}_hoistw{counter[0]}",
                            "opcode": "EventSemaphore",
                            "outs": [],
                            "sync_info": {"on_update": [],
                                          "on_wait": extra[i: i + limit]},
                        })
                new_insts.append(inst)
            bb["instructions"] = new_insts
    return json.dumps(j).encode()


def _build_program():
    if "prog" in _cache:
        return _cache["prog"]

    import concourse.bass as bass
    import concourse.tile as tile
    from concourse import mybir

    f32 = mybir.dt.float32
    u8 = mybir.dt.uint8
    nc = bass.Bass()
    masks_in = nc.dram_tensor("masks", [C, ROWS_PER_CORE, IMG_W], f32,
                              kind="ExternalInput")
    out_dram = nc.dram_tensor("out", [C, ROWS_PER_CORE, IMG_W], u8,
                              kind="ExternalOutput")

    bf16 = mybir.dt.bfloat16
    with tile.TileContext(nc) as tc:
        with (
            tc.tile_pool(name="inp", bufs=3) as inp,
            tc.tile_pool(name="cvt", bufs=2) as cvt,
            tc.tile_pool(name="outp", bufs=2) as outp,
            tc.tile_pool(name="scr", bufs=1) as scr,
        ):
            eqbig = None
            for t in range(NTILES):
                sl = slice(G * t, G * (t + 1))
                in_tile = inp.tile([128, C, G], f32, tag="in_tile")
                nc.sync.dma_start(
                    in_tile[:], masks_in[:, :, sl].rearrange("d p g -> p d g"))

                # f32 -> bf16 on the otherwise-idle scalar engine (monotonic
                # rounding preserves order; equal-after-round ties are patched
                # on the host); bf16 doubles DVE tensor_tensor throughput
                in16 = cvt.tile([128, C, G], bf16, tag="in16")
                nc.scalar.activation(in16[:], in_tile[:],
                                     mybir.ActivationFunctionType.Identity)

                # 5-step contiguous max tree over the channel axis
                s = scr.tile([128, 16, G], bf16, tag="tree")
                nc.vector.tensor_tensor(
                    out=s[:], in0=in16[:, 0:16, :], in1=in16[:, 16:32, :],
                    op=mybir.AluOpType.max)
                for h in (8, 4, 2, 1):
                    nc.vector.tensor_tensor(
                        out=s[:, 0:h, :], in0=s[:, 0:h, :], in1=s[:, h:2 * h, :],
                        op=mybir.AluOpType.max)

                # one-hot: compare every channel against the broadcast max;
                # pair two column tiles into one u8 out tile so the output
                # DMA writes 512B lines (no sub-512B RMW penalty)
                if t % 2 == 0:
                    eqbig = outp.tile([128, C, 2 * G], u8, tag="eq")
                half = slice((t % 2) * G, (t % 2 + 1) * G)
                mx_ap = s[:, 0, :]
                mx_b = bass.AP(tensor=mx_ap.tensor, offset=mx_ap.offset,
                               ap=[mx_ap.ap[0], [0, C], mx_ap.ap[-1]])
                nc.vector.tensor_tensor(out=eqbig[:, :, half], in0=in16[:],
                                        in1=mx_b, op=mybir.AluOpType.is_equal)

                if t % 2 == 1:
                    osl = slice(2 * G * (t // 2), 2 * G * (t // 2 + 1))
                    nc.sync.dma_start(
                        out_dram[:, :, osl].rearrange("c p g -> p c g"),
                        eqbig[:])

    orig = nc.to_json_bytes
    nc.to_json_bytes = lambda: _split_excess_waits(orig())
    _cache["prog"] = nc
    return nc


def kernel(masks, slot_features, pad_left, pad_top):
    from concourse.bass_utils import run_bass_kernel_spmd

    masks = np.asarray(masks, np.float32)
    slot_features = np.asarray(slot_features, np.float32)
    pl = [int(v) for v in np.asarray(pad_left)]
    pt = [int(v) for v in np.asarray(pad_top)]

    remap = _compute_remap(masks, slot_features, pl, pt)

    nc = _build_program()
    in_maps = []
    for i in range(N_CORES):
        slab = np.ascontiguousarray(
            masks[0, :, i * ROWS_PER_CORE:(i + 1) * ROWS_PER_CORE, :])
        in_maps.append({"masks": slab})

    res = run_bass_kernel_spmd(nc, in_maps, core_ids=list(range(N_CORES)))

    # assemble the per-core bf16 one-hots as booleans
    oh = np.empty((C, IMG_H, IMG_W), np.bool_)
    for i, r in enumerate(res.results):
        arr = np.asarray(r["out"])
        nz = arr.view(np.uint16) != 0 if arr.itemsize == 2 else arr != 0
        oh[:, i * ROWS_PER_CORE:(i + 1) * ROWS_PER_CORE, :] = nz

    # pixels where two channels tied bit-exactly produced two 1s; find them
    # before the merge pass and patch from the raw input afterwards
    colsum = oh.sum(axis=0, dtype=np.int16)
    ties = np.argwhere(colsum != 1)

    # merge remap as channel-plane ops (exactly the reference's add+zero scan)
    for d in range(C):
        k = int(remap[d])
        if k != d:
            oh[k] |= oh[d]
            oh[d] = False

    if len(ties):
        ys, xs = ties[:, 0], ties[:, 1]
        w = np.argmax(masks[0][:, ys, xs], axis=0)
        oh[:, ys, xs] = False
        oh[np.asarray(remap)[w], ys, xs] = True

    return oh.astype(np.float32)[None]
